# revision 5
# baseline (speedup 1.0000x reference)
"""Trainium2 Bass kernel for nn_Centroids (vq_codebook).

Computation (reference semantics):
  feat        = l1norm(feature)                                  [B, D]
  feature_new = l1norm(0.9 * feature_bank[ids] + 0.1 * feat)     [B, D]
  bank.at[ids].set(feature_new)          (last occurrence wins)
  sim         = cluster_means . feature_new  - 1e4 * wrong_class [Y, K, B]
  new_assigns = argmax_{yk} sim                                  [B]
  assigns.at[ids].set(new_assigns)
  corrects.at[ids].set(argmax(out, 1) == target)
  losses.at[ids].set(cross_entropy(out, target))

Sharding: feature_bank / assigns / corrects / losses are split row-wise
across 8 cores (25000 rows each).  Each (ids, feature) update is routed on
host to the core that owns the target row; cluster_means is replicated.
Each core copies its bank shard input->output (the dominant, bandwidth-bound
cost, ~51 MB each way) and scatters its ~1k updated rows on top.
"""

import numpy as np

import concourse.bacc as bacc
import concourse.bass as bass
import concourse.mybir as mybir
import concourse.tile as tile
from concourse.bass_utils import run_bass_kernel_spmd

F32 = mybir.dt.float32
I32 = mybir.dt.int32
AF = mybir.ActivationFunctionType
OP = mybir.AluOpType
AX = mybir.AxisListType

N, D, B, Y, K = 200000, 512, 8192, 10, 8
YK = Y * K
NCORES = 8
SHARD = N // NCORES  # 25000
DUMP = SHARD         # scratch row for dropped (duplicate-loser / padding) updates
MOM = 0.1
EPS = 1e-12
P = 128


def _build(M_cap: int):
    """Build the per-core Bass program (same program on all 8 cores)."""
    T = M_cap // P
    nc = bacc.Bacc("TRN2", target_bir_lowering=False, debug=False,
                   num_devices=NCORES)

    bank_in = nc.dram_tensor("bank_in", [SHARD, D], F32, kind="ExternalInput")
    assigns_in = nc.dram_tensor("assigns_in", [SHARD, 1], I32, kind="ExternalInput")
    corrects_in = nc.dram_tensor("corrects_in", [SHARD, 1], I32, kind="ExternalInput")
    losses_in = nc.dram_tensor("losses_in", [SHARD, 1], F32, kind="ExternalInput")
    feat_in = nc.dram_tensor("feat_in", [M_cap, D], F32, kind="ExternalInput")
    out_in = nc.dram_tensor("out_in", [M_cap, 16], F32, kind="ExternalInput")
    tgt_in = nc.dram_tensor("tgt_in", [M_cap, 1], F32, kind="ExternalInput")
    loc_in = nc.dram_tensor("loc_in", [M_cap, 1], I32, kind="ExternalInput")
    scat_in = nc.dram_tensor("scat_in", [M_cap, 1], I32, kind="ExternalInput")
    cmt_in = nc.dram_tensor("cmt_in", [D, YK], F32, kind="ExternalInput")
    cls_in = nc.dram_tensor("cls_in", [P, YK], F32, kind="ExternalInput")
    wgt_in = nc.dram_tensor("wgt_in", [P, YK], F32, kind="ExternalInput")
    iota16_in = nc.dram_tensor("iota16_in", [P, 16], F32, kind="ExternalInput")
    wgt16_in = nc.dram_tensor("wgt16_in", [P, 16], F32, kind="ExternalInput")

    bank_out = nc.dram_tensor("bank_out", [SHARD + 1, D], F32, kind="ExternalOutput")
    assigns_out = nc.dram_tensor("assigns_out", [SHARD + 1, 1], I32, kind="ExternalOutput")
    corrects_out = nc.dram_tensor("corrects_out", [SHARD + 1, 1], I32, kind="ExternalOutput")
    losses_out = nc.dram_tensor("losses_out", [SHARD + 1, 1], F32, kind="ExternalOutput")
    na_out = nc.dram_tensor("na_out", [M_cap, 1], I32, kind="ExternalOutput")

    with tile.TileContext(nc) as tc:
        with (
            tc.tile_pool(name="const", bufs=1) as cpool,
            tc.tile_pool(name="work", bufs=3) as pool,
            tc.tile_pool(name="small", bufs=4) as spool,
            tc.tile_pool(name="psum", bufs=2, space="PSUM") as pp,
        ):
            # ------- bulk shard copies (the bandwidth roofline) -------
            NCH = 4
            rows = SHARD // NCH
            for i in range(NCH):
                sl = slice(i * rows, (i + 1) * rows)
                nc.sync.dma_start(out=bank_out[sl, :], in_=bank_in[sl, :])
            nc.sync.dma_start(out=assigns_out[0:SHARD, :], in_=assigns_in[:, :])
            nc.sync.dma_start(out=corrects_out[0:SHARD, :], in_=corrects_in[:, :])
            nc.sync.dma_start(out=losses_out[0:SHARD, :], in_=losses_in[:, :])

            # ------- constants -------
            cmt = cpool.tile([P, 4 * YK], F32)   # chunk q at cols [q*80, (q+1)*80)
            for q in range(4):
                nc.sync.dma_start(out=cmt[:, q * YK:(q + 1) * YK],
                                  in_=cmt_in[q * P:(q + 1) * P, :])
            clsc = cpool.tile([P, YK], F32)
            nc.sync.dma_start(out=clsc[:], in_=cls_in[:])
            wgtc = cpool.tile([P, YK], F32)
            nc.sync.dma_start(out=wgtc[:], in_=wgt_in[:])
            iota16 = cpool.tile([P, 16], F32)
            nc.sync.dma_start(out=iota16[:], in_=iota16_in[:])
            wgt16 = cpool.tile([P, 16], F32)
            nc.sync.dma_start(out=wgt16[:], in_=wgt16_in[:])

            for t in range(T):
                sl = slice(t * P, (t + 1) * P)
                feat = pool.tile([P, D], F32)
                nc.sync.dma_start(out=feat[:], in_=feat_in[sl, :])
                loc = spool.tile([P, 1], I32)
                nc.sync.dma_start(out=loc[:], in_=loc_in[sl, :])
                scat = spool.tile([P, 1], I32)
                nc.sync.dma_start(out=scat[:], in_=scat_in[sl, :])
                tgt = spool.tile([P, 1], F32)
                nc.sync.dma_start(out=tgt[:], in_=tgt_in[sl, :])
                o_t = spool.tile([P, 16], F32)
                nc.sync.dma_start(out=o_t[:], in_=out_in[sl, :])

                gath = pool.tile([P, D], F32)
                nc.gpsimd.indirect_dma_start(
                    out=gath[:], out_offset=None, in_=bank_in[:, :],
                    in_offset=bass.IndirectOffsetOnAxis(ap=loc[:, :1], axis=0))

                # feat * (0.1 / max(sum|feat|, eps))
                absum = spool.tile([P, 1], F32)
                nc.vector.tensor_reduce(out=absum[:], in_=feat[:], op=OP.add,
                                        axis=AX.X, apply_absolute_value=True)
                nc.vector.tensor_scalar_max(out=absum[:], in0=absum[:], scalar1=EPS)
                rec = spool.tile([P, 1], F32)
                nc.vector.reciprocal(out=rec[:], in_=absum[:])
                nc.vector.tensor_scalar_mul(out=rec[:], in0=rec[:], scalar1=MOM)
                fn01 = pool.tile([P, D], F32)
                nc.scalar.activation(out=fn01[:], in_=feat[:], func=AF.Copy,
                                     bias=0.0, scale=rec[:, :1])

                # ema = 0.9 * bank[ids] + fn01 ; fnew = l1norm(ema)
                ema = pool.tile([P, D], F32)
                nc.vector.tensor_scalar_mul(out=ema[:], in0=gath[:], scalar1=1.0 - MOM)
                nc.vector.tensor_add(out=ema[:], in0=ema[:], in1=fn01[:])
                absum2 = spool.tile([P, 1], F32)
                nc.vector.tensor_reduce(out=absum2[:], in_=ema[:], op=OP.add,
                                        axis=AX.X, apply_absolute_value=True)
                nc.vector.tensor_scalar_max(out=absum2[:], in0=absum2[:], scalar1=EPS)
                rec2 = spool.tile([P, 1], F32)
                nc.vector.reciprocal(out=rec2[:], in_=absum2[:])
                fnew = pool.tile([P, D], F32)
                nc.scalar.activation(out=fnew[:], in_=ema[:], func=AF.Copy,
                                     bias=0.0, scale=rec2[:, :1])

                # scatter updated bank rows (losers/padding -> DUMP row)
                nc.gpsimd.indirect_dma_start(
                    out=bank_out[:, :],
                    out_offset=bass.IndirectOffsetOnAxis(ap=scat[:, :1], axis=0),
                    in_=fnew[:], in_offset=None)

                # sim[b, yk] = fnew . cm.  fnew must be transposed (d on
                # partitions) for the PE contraction; PE-transpose interleaved
                # with fp32 matmuls corrupts results on HW, so transpose on
                # DVE in 32x32 blocks: block (i,j) -> (j,i), fused over the
                # four 128-chunks per call.
                fnT = pool.tile([P, D], F32)
                fn3 = fnew[:].rearrange("p (q c) -> p q c", c=P)
                fT3 = fnT[:].rearrange("p (q c) -> p q c", c=P)
                for i in range(4):
                    for j in range(4):
                        nc.vector.transpose(
                            out=fT3[j * 32:(j + 1) * 32, :, i * 32:(i + 1) * 32],
                            in_=fn3[i * 32:(i + 1) * 32, :, j * 32:(j + 1) * 32])
                psim = pp.tile([P, YK], F32)
                for q in range(4):
                    nc.tensor.matmul(out=psim[:], lhsT=fnT[:, q * P:(q + 1) * P],
                                     rhs=cmt[:, q * YK:(q + 1) * YK],
                                     start=(q == 0), stop=(q == 3))

                # mask wrong classes: sim += 1e4*(cls == target) - 1e4
                eq = spool.tile([P, YK], F32)
                nc.vector.tensor_scalar(out=eq[:], in0=clsc[:], scalar1=tgt[:, :1],
                                        scalar2=None, op0=OP.is_equal)
                nc.vector.tensor_scalar(out=eq[:], in0=eq[:], scalar1=1e4,
                                        scalar2=-1e4, op0=OP.mult, op1=OP.add)
                simm = spool.tile([P, YK], F32)
                nc.vector.tensor_add(out=simm[:], in0=psim[:], in1=eq[:])

                # argmax over 80 (first max wins): idx = 128 - max((sim==max)*(128-j))
                mx = spool.tile([P, 1], F32)
                nc.vector.tensor_reduce(out=mx[:], in_=simm[:], op=OP.max, axis=AX.X)
                nc.vector.tensor_scalar(out=simm[:], in0=simm[:], scalar1=mx[:, :1],
                                        scalar2=None, op0=OP.is_equal)
                nc.vector.tensor_mul(out=simm[:], in0=simm[:], in1=wgtc[:])
                nc.vector.tensor_reduce(out=mx[:], in_=simm[:], op=OP.max, axis=AX.X)
                naf = spool.tile([P, 1], F32)
                nc.vector.tensor_scalar(out=naf[:], in0=mx[:], scalar1=-1.0,
                                        scalar2=float(P), op0=OP.mult, op1=OP.add)
                nai = spool.tile([P, 1], I32)
                nc.vector.tensor_copy(out=nai[:], in_=naf[:])
                nc.sync.dma_start(out=na_out[sl, :], in_=nai[:])
                nc.gpsimd.indirect_dma_start(
                    out=assigns_out[:, :],
                    out_offset=bass.IndirectOffsetOnAxis(ap=scat[:, :1], axis=0),
                    in_=nai[:], in_offset=None)

                # cross entropy: ce = max + log(sum exp(o - max)) - o[target]
                mo = spool.tile([P, 1], F32)
                nc.vector.tensor_reduce(out=mo[:], in_=o_t[:], op=OP.max, axis=AX.X)
                mneg = spool.tile([P, 1], F32)
                nc.vector.tensor_scalar_mul(out=mneg[:], in0=mo[:], scalar1=-1.0)
                expd = spool.tile([P, 16], F32)
                sexp = spool.tile([P, 1], F32)
                nc.scalar.activation(out=expd[:], in_=o_t[:], func=AF.Exp,
                                     bias=mneg[:, :1], scale=1.0,
                                     accum_out=sexp[:, :1])
                lse = spool.tile([P, 1], F32)
                nc.scalar.activation(out=lse[:], in_=sexp[:], func=AF.Ln)
                eq16 = spool.tile([P, 16], F32)
                nc.vector.tensor_scalar(out=eq16[:], in0=iota16[:], scalar1=tgt[:, :1],
                                        scalar2=None, op0=OP.is_equal)
                pick = spool.tile([P, 1], F32)
                dmy = spool.tile([P, 16], F32)
                nc.vector.tensor_mul(out=dmy[:], in0=eq16[:], in1=o_t[:])
                nc.vector.tensor_reduce(out=pick[:], in_=dmy[:], op=OP.add, axis=AX.X)
                ce = spool.tile([P, 1], F32)
                nc.vector.tensor_add(out=ce[:], in0=mo[:], in1=lse[:])
                nc.vector.tensor_sub(out=ce[:], in0=ce[:], in1=pick[:])
                nc.gpsimd.indirect_dma_start(
                    out=losses_out[:, :],
                    out_offset=bass.IndirectOffsetOnAxis(ap=scat[:, :1], axis=0),
                    in_=ce[:], in_offset=None)

                # corrects: argmax(out) == target
                eqo = spool.tile([P, 16], F32)
                nc.vector.tensor_scalar(out=eqo[:], in0=o_t[:], scalar1=mo[:, :1],
                                        scalar2=None, op0=OP.is_equal)
                nc.vector.tensor_mul(out=eqo[:], in0=eqo[:], in1=wgt16[:])
                amx = spool.tile([P, 1], F32)
                nc.vector.tensor_reduce(out=amx[:], in_=eqo[:], op=OP.max, axis=AX.X)
                nc.vector.tensor_scalar(out=amx[:], in0=amx[:], scalar1=-1.0,
                                        scalar2=16.0, op0=OP.mult, op1=OP.add)
                corrf = spool.tile([P, 1], F32)
                nc.vector.tensor_scalar(out=corrf[:], in0=amx[:], scalar1=tgt[:, :1],
                                        scalar2=None, op0=OP.is_equal)
                corri = spool.tile([P, 1], I32)
                nc.vector.tensor_copy(out=corri[:], in_=corrf[:])
                nc.gpsimd.indirect_dma_start(
                    out=corrects_out[:, :],
                    out_offset=bass.IndirectOffsetOnAxis(ap=scat[:, :1], axis=0),
                    in_=corri[:], in_offset=None)

    nc.compile()
    return nc


def _route(ids: np.ndarray):
    """Host-side routing: group batch entries by owning shard; mark winners
    (last occurrence of each id, matching sequential scatter semantics)."""
    shard_of = ids // SHARD
    order = np.argsort(shard_of, kind="stable")
    counts = np.bincount(shard_of, minlength=NCORES)
    offs = np.concatenate([[0], np.cumsum(counts)])
    win = np.zeros(B, dtype=bool)
    _, first_rev = np.unique(ids[::-1], return_index=True)
    win[B - 1 - first_rev] = True
    return order, counts, offs, win


def kernel(**inputs) -> tuple:
    feature = np.ascontiguousarray(np.asarray(inputs["feature"], np.float32))
    out = np.ascontiguousarray(np.asarray(inputs["out"], np.float32))
    bank = np.ascontiguousarray(np.asarray(inputs["feature_bank"], np.float32))
    cm = np.ascontiguousarray(np.asarray(inputs["cluster_means"], np.float32))
    losses = np.ascontiguousarray(np.asarray(inputs["losses_state"], np.float32))
    target = np.asarray(inputs["target"], np.int32)
    ids = np.asarray(inputs["ids"], np.int32)
    assigns = np.ascontiguousarray(np.asarray(inputs["assigns"], np.int32))
    corrects = np.ascontiguousarray(np.asarray(inputs["corrects_state"], np.int32))

    order, counts, offs, win = _route(ids)
    M_cap = max(int(-(-max(int(counts.max()), 1) // P)) * P, P)

    nc = _build(M_cap)
    in_maps = _make_in_maps(feature, out, bank, cm, losses, target, ids,
                            assigns, corrects, order, counts, offs, win, M_cap)
    res = run_bass_kernel_spmd(nc, in_maps, core_ids=list(range(NCORES))).results
    return _assemble(res, order, counts, offs)


def _make_in_maps(feature, out, bank, cm, losses, target, ids, assigns,
                  corrects, order, counts, offs, win, M_cap):
    cmt = np.ascontiguousarray(cm.reshape(YK, D).T)  # [D, YK]
    clsc = np.tile((np.arange(YK, dtype=np.float32) // K), (P, 1))
    wgtc = np.tile(P - np.arange(YK, dtype=np.float32), (P, 1))
    iota16 = np.tile(np.arange(16, dtype=np.float32), (P, 1))
    wgt16 = np.tile(16.0 - np.arange(16, dtype=np.float32), (P, 1))

    in_maps = []
    for c in range(NCORES):
        cnt = int(counts[c])
        sel = order[offs[c]:offs[c] + cnt]
        feat_c = np.zeros((M_cap, D), np.float32)
        feat_c[:cnt] = feature[sel]
        out_c = np.full((M_cap, 16), -1e30, np.float32)
        out_c[:cnt, :Y] = out[sel]
        tgt_c = np.zeros(M_cap, np.float32)
        tgt_c[:cnt] = target[sel]
        loc_c = np.zeros(M_cap, np.int32)
        loc_c[:cnt] = ids[sel] - c * SHARD
        scat_c = np.full(M_cap, DUMP, np.int32)
        scat_c[:cnt] = np.where(win[sel], loc_c[:cnt], DUMP)
        in_maps.append({
            "bank_in": bank[c * SHARD:(c + 1) * SHARD],
            "assigns_in": assigns[c * SHARD:(c + 1) * SHARD, None],
            "corrects_in": corrects[c * SHARD:(c + 1) * SHARD, None],
            "losses_in": losses[c * SHARD:(c + 1) * SHARD, None],
            "feat_in": feat_c,
            "out_in": out_c,
            "tgt_in": tgt_c[:, None],
            "loc_in": loc_c[:, None],
            "scat_in": scat_c[:, None],
            "cmt_in": cmt,
            "cls_in": clsc,
            "wgt_in": wgtc,
            "iota16_in": iota16,
            "wgt16_in": wgt16,
        })
    return in_maps


def _assemble(res, order, counts, offs):
    bank_new = np.concatenate([res[c]["bank_out"][:SHARD] for c in range(NCORES)])
    assigns_new = np.concatenate([res[c]["assigns_out"][:SHARD, 0] for c in range(NCORES)])
    corrects_new = np.concatenate([res[c]["corrects_out"][:SHARD, 0] for c in range(NCORES)])
    losses_new = np.concatenate([res[c]["losses_out"][:SHARD, 0] for c in range(NCORES)])
    new_assigns = np.empty(B, np.int32)
    for c in range(NCORES):
        cnt = int(counts[c])
        sel = order[offs[c]:offs[c] + cnt]
        new_assigns[sel] = res[c]["na_out"][:cnt, 0]
    return bank_new, assigns_new, corrects_new, losses_new, new_assigns
